# revision 12
# baseline (speedup 1.0000x reference)
"""MoE BasicBlock kernel v3 for TRN2: 1D Winograd F(2,3) along x.

W = sum_e alpha_e * w_e (21 experts), conv3x3 -> BN -> relu -> conv3x3 -> BN
-> +x -> relu on x [N,256,56,56] f32. Data-parallel across 8 cores (4
images/core); the alpha-combine is oc-sharded and shared via one fused bf16
AllGather per rep (as v2).

v3 replaces direct conv with 1D Winograd F(2,3) along the x axis, cutting PE
multiplies 1.5x (18 -> 12 effective K=128 matmul-equivalents per output):
  per output row-pair (x = 2t, 2t+1) and ky tap:
    d = [xp[2t], xp[2t+1], xp[2t+2], xp[2t+3]]  (padded cols)
    U0 = d0-d2, U1 = d1+d2, U2 = d1-d2, U3 = d1-d3         (moving, bf16)
    G0 = w0, G1 = (w0+w1+w2)/2, G2 = (w1-w0-w2)/2, G3 = w2 (stationary; G2
         carries the folded sign so psum slab nu=2 equals Lavin's m3)
    y0 = m0+m1+m2 ; y1 = m1-m2-m3                          (DVE, f32 psum)
Layout tricks:
  - activations live as even/odd padded-column planes ([128,58,29] bf16) so
    every U combine is a packed stride-1 read (DVE 2x eligible) and the
    conv2 residual add reads packed slices;
  - psum tiles are [128,2,512] f32 (two banks; each nu slab bank-aligned so
    interleaved accumulation groups never share a zero-region);
  - the expert combine is emitted as a generator pumped between row-tile
    blocks so no long DVE block starves psum drains;
  - U builds run on GPSIMD for conv1 and DVE for conv2; drains on DVE;
    epilogues on ACT.
"""

import numpy as np

import concourse.bass as bass
import concourse.mybir as mybir
import concourse.tile as tile
from concourse.masks import make_identity

FP32 = mybir.dt.float32
BF16 = mybir.dt.bfloat16

C = 256  # channels
CCH = 2  # channel chunks of 128
H = W = 56
HP = 58  # padded rows
PC = 29  # cols per even/odd plane
TW = 28  # winograd output col-pairs
E = 21  # experts
KHW = 9
IC9 = C * KHW  # 2304
RT = (14, 14, 14, 14)  # output rows per tile (56 = 4*14)
R0 = (0, 14, 28, 42)
NRT = 4
EPS = 1e-5
COPY = mybir.ActivationFunctionType.Copy
RELU = mybir.ActivationFunctionType.Relu
MUL = mybir.AluOpType.mult
ADD = mybir.AluOpType.add


def split_multi_waits(nc):
    """The installed walrus accepts at most one sync-wait per instruction
    (two for EventSemaphore). Tile's sem assignment can emit more; split the
    extras onto injected same-engine nops placed immediately before the
    offending instruction (equivalent semantics for in-order engine streams).
    """
    n_split = 0
    n_dma_split = 0
    for bb in nc.main_func.blocks:
        new_list = []
        for inst in list(bb.instructions):
            si = inst.sync_info
            waits = list(si.on_wait) if si is not None and si.on_wait else []
            cap = 2 if isinstance(inst, mybir.InstEventSemaphore) else 1
            if len(waits) > cap:
                if getattr(inst, "queue", None) is not None:
                    n_dma_split += 1
                extra, keep = waits[:-cap], waits[-cap:]
                for w in extra:
                    nop = nc.engines[inst.engine].nop(hint="waitsplit", nofuse=True)
                    host_bb = nc.cur_bb
                    assert host_bb.bb.instructions[-1] is nop.ins
                    host_bb.bb.instructions.pop()
                    nop.ins.sync_info = mybir.SyncInfo(on_update=[], on_wait=[w])
                    new_list.append(nop.ins)
                    n_split += 1
                inst.sync_info = mybir.SyncInfo(
                    on_update=list(si.on_update) if si.on_update else [], on_wait=keep
                )
            new_list.append(inst)
        bb.instructions[:] = new_list
    return n_split, n_dma_split


def build_nc(npc=4, repeat=1, n_cores=8):
    nc = bass.Bass(
        "TRN2", target_bir_lowering=False, debug=False, num_devices=n_cores
    )

    OSH = C // n_cores  # oc rows combined per core (32)
    SFREE = OSH * IC9 // 128  # 576

    x = nc.dram_tensor("x", [npc, C, H, W], FP32, kind="ExternalInput")
    alpha = nc.dram_tensor("alpha", [E], FP32, kind="ExternalInput")
    w1 = nc.dram_tensor("w1s", [E, OSH, C, 3, 3], FP32, kind="ExternalInput")
    w2 = nc.dram_tensor("w2s", [E, OSH, C, 3, 3], FP32, kind="ExternalInput")
    bn = {}
    for nm in ("g1", "b1", "m1", "v1", "g2", "b2", "m2", "v2"):
        bn[nm] = nc.dram_tensor(nm, [C], FP32, kind="ExternalInput")
    out = nc.dram_tensor("out", [npc, C, H, W], FP32, kind="ExternalOutput")

    xap = x.ap().rearrange("n c h w -> n c (h w)")
    oap = out.ap().rearrange("n c h w -> n c (h w)")
    w1ap = w1.ap().rearrange("e o i h w -> e (o i h w)")
    w2ap = w2.ap().rearrange("e o i h w -> e (o i h w)")
    wparts = [nc.dram_tensor(f"wpart{i}", [2 * OSH * IC9], BF16) for i in range(2)]
    wgaths = [
        nc.dram_tensor(f"wgath{i}", [n_cores, 2, OSH, IC9], BF16, addr_space="Shared")
        for i in range(2)
    ]

    with tile.TileContext(nc) as tc:
        import contextlib

        with contextlib.ExitStack() as ctx:
            singles = ctx.enter_context(tc.tile_pool(name="singles", bufs=1))
            epool = ctx.enter_context(tc.tile_pool(name="epool", bufs=3))
            wfpool = ctx.enter_context(tc.tile_pool(name="wfpool", bufs=2))
            accpool = ctx.enter_context(tc.tile_pool(name="accpool", bufs=2))
            xspool = ctx.enter_context(tc.tile_pool(name="xspool", bufs=2))
            xplanes = ctx.enter_context(tc.tile_pool(name="xplanes", bufs=4 * npc))
            hplanes = ctx.enter_context(tc.tile_pool(name="hplanes", bufs=8))
            upool = ctx.enter_context(tc.tile_pool(name="upool", bufs=6))
            ypool = ctx.enter_context(tc.tile_pool(name="ypool", bufs=4))
            ytpool = ctx.enter_context(tc.tile_pool(name="ytpool", bufs=8))
            rrpool = ctx.enter_context(tc.tile_pool(name="rrpool", bufs=4))
            wscr = ctx.enter_context(tc.tile_pool(name="wscr", bufs=4))
            obpool = ctx.enter_context(tc.tile_pool(name="obpool", bufs=2))
            cpsum = ctx.enter_context(tc.tile_pool(name="cpsum", bufs=3, space="PSUM"))
            tpsum = ctx.enter_context(tc.tile_pool(name="tpsum", bufs=2, space="PSUM"))

            # ---- stage 0: BN params, alpha, identity ----
            ident = singles.tile([128, 128], BF16, tag="ident")
            make_identity(nc, ident[:])

            zero_c = singles.tile([128, 1], FP32, tag="zero_c")
            nc.vector.memset(zero_c[:], 0.0)
            nc.const_aps.aps[(FP32, 0.0)] = zero_c[:]
            eps_c = singles.tile([128, 1], FP32, tag="eps_c")
            nc.vector.memset(eps_c[:], EPS)

            alpha_sb = singles.tile([128, E], FP32, tag="alpha")
            nc.sync.dma_start(
                out=alpha_sb[:],
                in_=bass.AP(tensor=alpha.ap().tensor, offset=0, ap=[[0, 128], [1, E]]),
            )

            bns = {}
            for nm in ("g1", "b1", "m1", "v1", "g2", "b2", "m2", "v2"):
                t = singles.tile([128, CCH], FP32, name=f"bn_{nm}", tag=f"bn_{nm}")
                nc.sync.dma_start(
                    out=t[:],
                    in_=bass.AP(
                        tensor=bn[nm].ap().tensor, offset=0, ap=[[1, 128], [128, CCH]]
                    ),
                )
                bns[nm] = t

            def bn_fold(g, b, m, v, idx):
                sq = singles.tile([128, CCH], FP32, name=f"bn_sq{idx}", tag=f"bn_sq{idx}")
                nc.scalar.activation(
                    sq[:], v[:], mybir.ActivationFunctionType.Sqrt, bias=eps_c[:]
                )
                r = singles.tile([128, CCH], FP32, name=f"bn_r{idx}", tag=f"bn_r{idx}")
                nc.vector.reciprocal(r[:], sq[:])
                ve = singles.tile([128, CCH], FP32, name=f"bn_ve{idx}", tag=f"bn_ve{idx}")
                nc.vector.tensor_scalar_add(ve[:], v[:], EPS)
                t1 = singles.tile([128, CCH], FP32, name=f"bn_t1{idx}", tag=f"bn_t1{idx}")
                nc.vector.tensor_mul(t1[:], ve[:], r[:])
                nc.vector.tensor_add(t1[:], t1[:], sq[:])
                nc.vector.tensor_scalar_mul(t1[:], t1[:], 0.5)
                nc.vector.reciprocal(r[:], t1[:])
                s = singles.tile([128, CCH], FP32, name=f"bn_s{idx}", tag=f"bn_s{idx}")
                nc.vector.tensor_mul(s[:], g[:], r[:])
                bp = singles.tile([128, CCH], FP32, name=f"bn_bp{idx}", tag=f"bn_bp{idx}")
                nc.vector.tensor_mul(bp[:], m[:], s[:])
                nc.vector.tensor_sub(bp[:], b[:], bp[:])
                return s, bp

            s1, b1p = bn_fold(bns["g1"], bns["b1"], bns["m1"], bns["v1"], 1)
            s2, b2p = bn_fold(bns["g2"], bns["b2"], bns["m2"], bns["v2"], 2)

            # G-transformed stationary: [128ic, 3ky, 4nu, 128oc] bf16
            lhsT = [
                [
                    [
                        [
                            singles.tile(
                                [128, 3, 4, 128], BF16,
                                name=f"lhsT_{par}_{wi}_{ic}_{oc}",
                                tag=f"lhsT_{par}_{wi}_{ic}_{oc}",
                            )
                            for oc in range(CCH)
                        ]
                        for ic in range(CCH)
                    ]
                    for wi in range(2)
                ]
                for par in range(2)
            ]

            # ---- pumped weight combine + one fused bf16 AllGather ----
            gens = []

            def pump():
                while gens:
                    try:
                        next(gens[0])
                        return
                    except StopIteration:
                        gens.pop(0)

            def drain():
                while gens:
                    try:
                        next(gens[0])
                    except StopIteration:
                        gens.pop(0)

            def combine_gather_gen(par):
                accb = accpool.tile([128, 2 * SFREE], BF16, name="accb", tag="accb")
                for wi, wap in ((0, w1ap), (1, w2ap)):
                    acc = wfpool.tile([128, SFREE], FP32, name="sacc", tag="sacc")
                    for e0 in range(0, E, 3):
                        for e in range(e0, min(e0 + 3, E)):
                            est = epool.tile([128, SFREE], FP32, name="sest", tag="sest")
                            nc.sync.dma_start(
                                out=est[:],
                                in_=wap[e].rearrange("(p f) -> p f", p=128),
                            )
                            if e == 0:
                                nc.gpsimd.tensor_scalar_mul(
                                    acc[:], est[:], alpha_sb[:, 0:1]
                                )
                            else:
                                nc.gpsimd.scalar_tensor_tensor(
                                    acc[:], est[:], alpha_sb[:, e : e + 1], acc[:],
                                    op0=MUL, op1=ADD,
                                )
                        yield
                    nc.scalar.copy(
                        out=accb[:, wi * SFREE : (wi + 1) * SFREE], in_=acc[:]
                    )
                    yield
                nc.sync.dma_start(
                    out=wparts[par].ap().rearrange("(w p f) -> p w f", w=2, p=128),
                    in_=accb[:].rearrange("p (w f) -> p w f", w=2),
                )
                nc.gpsimd.collective_compute(
                    "AllGather",
                    mybir.AluOpType.bypass,
                    replica_groups=[list(range(n_cores))],
                    ins=[wparts[par].ap().opt()],
                    outs=[wgaths[par].ap().rearrange("c w o f -> (c w o f)").opt()],
                )

            def load_transpose(par, wi):
                # gathered rows of weight wi live at wgath[4*oc+k, wi, :, :]
                for oc in range(CCH):
                    wf = wfpool.tile([128, IC9], BF16, name="wfull", tag="wfull")
                    for k in range(4):
                        nc.sync.dma_start(
                            out=wf[32 * k : 32 * (k + 1), :],
                            in_=wgaths[par].ap()[4 * oc + k, wi],
                        )
                    wfr = wf[:].rearrange("p (c i r) -> p c i r", c=CCH, r=KHW)
                    for ic in range(CCH):
                        dst = lhsT[par][wi][ic][oc]
                        for ky in range(3):
                            # transpose the three kx taps, then G-combine:
                            # nu0 = w0, nu3 = w2, s1h = w1/2,
                            # nu1 = (w0+w2)/2 + s1h, nu2 = -(w0+w2)/2 + s1h
                            pt = tpsum.tile([128, 128], BF16, name="tp", tag="tp")
                            nc.tensor.transpose(
                                pt[:], wfr[:, ic, :, 3 * ky + 0], ident[:]
                            )
                            nc.scalar.copy(out=dst[:, ky, 0, :], in_=pt[:])
                            pt = tpsum.tile([128, 128], BF16, name="tp", tag="tp")
                            nc.tensor.transpose(
                                pt[:], wfr[:, ic, :, 3 * ky + 1], ident[:]
                            )
                            s1h = wscr.tile([128, 128], BF16, name="ws", tag="ws")
                            nc.scalar.activation(s1h[:], pt[:], COPY, scale=0.5)
                            pt = tpsum.tile([128, 128], BF16, name="tp", tag="tp")
                            nc.tensor.transpose(
                                pt[:], wfr[:, ic, :, 3 * ky + 2], ident[:]
                            )
                            nc.scalar.copy(out=dst[:, ky, 3, :], in_=pt[:])
                            qh = wscr.tile([128, 128], BF16, name="ws", tag="ws")
                            nc.vector.tensor_add(
                                qh[:], dst[:, ky, 0, :], dst[:, ky, 3, :]
                            )
                            nc.vector.scalar_tensor_tensor(
                                dst[:, ky, 1, :], qh[:], 0.5, s1h[:],
                                op0=MUL, op1=ADD,
                            )
                            nc.vector.scalar_tensor_tensor(
                                dst[:, ky, 2, :], qh[:], -0.5, s1h[:],
                                op0=MUL, op1=ADD,
                            )

            # ---- even/odd padded-column planes ----
            def fill_planes(n):
                planes = []
                for c in range(CCH):
                    xe = xplanes.tile([128, HP, PC], BF16, name="xpl", tag="xpl")
                    xo = xplanes.tile([128, HP, PC], BF16, name="xpl", tag="xpl")
                    nc.gpsimd.memset(xe[:, 0, :], 0.0)
                    nc.gpsimd.memset(xe[:, HP - 1, :], 0.0)
                    nc.gpsimd.memset(xe[:, 1 : HP - 1, 0:1], 0.0)
                    nc.gpsimd.memset(xo[:, 0, :], 0.0)
                    nc.gpsimd.memset(xo[:, HP - 1, :], 0.0)
                    nc.gpsimd.memset(xo[:, 1 : HP - 1, PC - 1 : PC], 0.0)
                    for half in range(2):
                        xst = xspool.tile([128, 28 * W], FP32, name="xst", tag="xst")
                        nc.sync.dma_start(
                            out=xst[:],
                            in_=xap[
                                n, c * 128 : (c + 1) * 128,
                                half * 28 * W : (half + 1) * 28 * W,
                            ],
                        )
                        xsr = xst[:].rearrange("p (r t two) -> p r t two", r=28, two=2)
                        r0 = 1 + 28 * half
                        nc.scalar.copy(
                            out=xe[:, r0 : r0 + 28, 1:PC], in_=xsr[:, :, :, 1]
                        )
                        nc.scalar.copy(
                            out=xo[:, r0 : r0 + 28, 0 : PC - 1], in_=xsr[:, :, :, 0]
                        )
                    planes.append((xe, xo))
                return planes

            def alloc_hplanes():
                sets = []
                for c in range(CCH):
                    he = hplanes.tile([128, HP, PC], BF16, name="hpl", tag="hpl")
                    ho = hplanes.tile([128, HP, PC], BF16, name="hpl", tag="hpl")
                    nc.gpsimd.memset(he[:, 0, :], 0.0)
                    nc.gpsimd.memset(he[:, HP - 1, :], 0.0)
                    nc.gpsimd.memset(he[:, 1 : HP - 1, 0:1], 0.0)
                    nc.gpsimd.memset(ho[:, 0, :], 0.0)
                    nc.gpsimd.memset(ho[:, HP - 1, :], 0.0)
                    nc.gpsimd.memset(ho[:, 1 : HP - 1, PC - 1 : PC], 0.0)
                    sets.append((he, ho))
                return sets

            def build_u(src, ic, rt, eng):
                r0 = R0[rt]
                rows = RT[rt] + 2
                pe_, po = src[ic]
                u = upool.tile([128, 4, 16, 28], BF16, name="u", tag="u")
                a = pe_[:, r0 : r0 + rows, 0:28]
                b = pe_[:, r0 : r0 + rows, 1:29]
                cc = po[:, r0 : r0 + rows, 0:28]
                dd = po[:, r0 : r0 + rows, 1:29]
                eng.tensor_sub(u[:, 0, :rows, :], a, b)
                eng.tensor_add(u[:, 1, :rows, :], cc, b)
                eng.tensor_sub(u[:, 2, :rows, :], cc, b)
                eng.tensor_sub(u[:, 3, :rows, :], cc, dd)
                return u

            def conv(n, src, wi, dst, par):
                """One Winograd conv over image n. src: per-chunk (even, odd)
                planes. dst: wi=0 -> hplane sets; wi=1 -> x planes (residual),
                output DMA'd."""
                eng_u = nc.gpsimd if wi == 0 else nc.vector
                us = [build_u(src, ic, 0, eng_u) for ic in range(CCH)]
                for rt in range(NRT):
                    rows = RT[rt]
                    r0 = R0[rt]
                    fcols = rows * TW
                    cur = us
                    for oc in range(CCH):
                        ps01 = cpsum.tile([128, 2, 512], FP32, name="cp", tag="cp")
                        ps23 = cpsum.tile([128, 2, 512], FP32, name="cp", tag="cp")
                        for ic in range(CCH):
                            for ky in range(3):
                                for nu in range(4):
                                    pst = ps01 if nu < 2 else ps23
                                    nc.tensor.matmul(
                                        pst[:, nu % 2, 0:fcols],
                                        lhsT[par][wi][ic][oc][:, ky, nu, :],
                                        cur[ic][:, nu, ky : ky + rows, :],
                                        start=(ic == 0 and ky == 0),
                                        stop=(ic == 1 and ky == 2),
                                    )
                        if oc == 0 and rt + 1 < NRT:
                            # next row-tile's U ahead of the drains in the
                            # u-engine stream
                            us = [
                                build_u(src, ic, rt + 1, eng_u) for ic in range(CCH)
                            ]
                        m0 = ps01[:, 0, 0:fcols]
                        m1 = ps01[:, 1, 0:fcols]
                        m2 = ps23[:, 0, 0:fcols]
                        m3 = ps23[:, 1, 0:fcols]
                        y0 = ypool.tile([128, 14, 28], BF16, name="y", tag="y")
                        y1 = ypool.tile([128, 14, 28], BF16, name="y", tag="y")
                        c0 = ytpool.tile([128, 14, 28], BF16, name="yt", tag="yt")
                        c2 = ytpool.tile([128, 14, 28], BF16, name="yt", tag="yt")
                        t0 = ytpool.tile([128, 14, 28], BF16, name="yt", tag="yt")
                        t1 = ytpool.tile([128, 14, 28], BF16, name="yt", tag="yt")
                        y0f = y0[:].rearrange("p r c -> p (r c)")[:, 0:fcols]
                        y1f = y1[:].rearrange("p r c -> p (r c)")[:, 0:fcols]
                        c0f = c0[:].rearrange("p r c -> p (r c)")[:, 0:fcols]
                        c2f = c2[:].rearrange("p r c -> p (r c)")[:, 0:fcols]
                        t0f = t0[:].rearrange("p r c -> p (r c)")[:, 0:fcols]
                        t1f = t1[:].rearrange("p r c -> p (r c)")[:, 0:fcols]
                        # TensorTensor may read only one PSUM operand: stage
                        # m1/m2 to SBUF on ACT so two of the four DVE combines
                        # run on packed bf16 SBUF operands (2x eligible).
                        nc.scalar.copy(out=c0f, in_=m1)
                        nc.scalar.copy(out=c2f, in_=m2)
                        nc.vector.tensor_add(t0f, m0, c0f)
                        nc.vector.tensor_add(y0f, t0f, c2f)
                        nc.vector.tensor_sub(t1f, c0f, c2f)
                        nc.vector.tensor_sub(y1f, t1f, m3)
                        if wi == 0:
                            he, ho = dst[oc]
                            nc.scalar.activation(
                                ho[:, r0 + 1 : r0 + 1 + rows, 0:28],
                                y0[:, :rows, :], RELU,
                                bias=b1p[:, oc : oc + 1], scale=s1[:, oc : oc + 1],
                            )
                            nc.scalar.activation(
                                he[:, r0 + 1 : r0 + 1 + rows, 1:29],
                                y1[:, :rows, :], RELU,
                                bias=b1p[:, oc : oc + 1], scale=s1[:, oc : oc + 1],
                            )
                        else:
                            xe, xo = dst[oc]
                            rr0 = rrpool.tile([128, 14, 28], BF16, name="rr", tag="rr")
                            rr1 = rrpool.tile([128, 14, 28], BF16, name="rr", tag="rr")
                            nc.vector.scalar_tensor_tensor(
                                rr0[:, :rows, :], y0[:, :rows, :],
                                s2[:, oc : oc + 1],
                                xo[:, r0 + 1 : r0 + 1 + rows, 0:28],
                                op0=MUL, op1=ADD,
                            )
                            nc.vector.scalar_tensor_tensor(
                                rr1[:, :rows, :], y1[:, :rows, :],
                                s2[:, oc : oc + 1],
                                xe[:, r0 + 1 : r0 + 1 + rows, 1:29],
                                op0=MUL, op1=ADD,
                            )
                            ob = obpool.tile([128, 14, 56], FP32, name="ob", tag="ob")
                            obv = ob[:].rearrange("p r (t two) -> p r t two", two=2)
                            nc.scalar.activation(
                                obv[:, :rows, :, 0], rr0[:, :rows, :], RELU,
                                bias=b2p[:, oc : oc + 1],
                            )
                            nc.scalar.activation(
                                obv[:, :rows, :, 1], rr1[:, :rows, :], RELU,
                                bias=b2p[:, oc : oc + 1],
                            )
                            nc.sync.dma_start(
                                out=oap[
                                    n, oc * 128 : (oc + 1) * 128,
                                    r0 * W : (r0 + rows) * W,
                                ],
                                in_=ob[:].rearrange("p r c -> p (r c)")[
                                    :, 0 : rows * W
                                ],
                            )
                    pump()

            # ---- emission schedule ----
            assert npc == 4
            gens.append(combine_gather_gen(0))
            drain()
            load_transpose(0, 0)
            load_transpose(0, 1)
            for rep in range(repeat):
                par = rep % 2
                if rep + 1 < repeat:
                    gens.append(combine_gather_gen(1 - par))
                xps = {0: fill_planes(0), 1: fill_planes(1)}
                hps = {}
                hps[0] = alloc_hplanes()
                conv(0, xps[0], 0, hps[0], par)
                xps[2] = fill_planes(2)
                hps[1] = alloc_hplanes()
                conv(1, xps[1], 0, hps[1], par)
                conv(0, hps[0], 1, xps[0], par)
                xps[3] = fill_planes(3)
                hps[2] = alloc_hplanes()
                conv(2, xps[2], 0, hps[2], par)
                drain()
                if rep + 1 < repeat:
                    load_transpose(1 - par, 0)
                conv(1, hps[1], 1, xps[1], par)
                hps[3] = alloc_hplanes()
                conv(3, xps[3], 0, hps[3], par)
                if rep + 1 < repeat:
                    load_transpose(1 - par, 1)
                conv(2, hps[2], 1, xps[2], par)
                conv(3, hps[3], 1, xps[3], par)

    n_split, n_dma_split = split_multi_waits(nc)
    return nc, (n_split, n_dma_split)


# ---------------------------------------------------------------------------
# Host-side entry point: takes FULL inputs, shards batch across 8 cores.
# ---------------------------------------------------------------------------
_NC_CACHE = {}


def kernel(**inputs):
    from concourse.bass_utils import run_bass_kernel_spmd

    x = np.ascontiguousarray(np.asarray(inputs["x"], dtype=np.float32))
    n_total = x.shape[0]
    n_cores = 8
    npc = n_total // n_cores
    assert npc * n_cores == n_total

    key = npc
    if key not in _NC_CACHE:
        _NC_CACHE[key] = build_nc(npc=npc)[0]
    nc = _NC_CACHE[key]

    w1 = np.asarray(inputs["w1"], dtype=np.float32)
    w2 = np.asarray(inputs["w2"], dtype=np.float32)
    osh = w1.shape[1] // n_cores
    shared = {
        k: np.ascontiguousarray(np.asarray(v, dtype=np.float32))
        for k, v in inputs.items()
        if k not in ("x", "w1", "w2")
    }
    in_maps = [
        {
            "x": x[c * npc : (c + 1) * npc],
            "w1s": np.ascontiguousarray(w1[:, c * osh : (c + 1) * osh]),
            "w2s": np.ascontiguousarray(w2[:, c * osh : (c + 1) * osh]),
            **shared,
        }
        for c in range(n_cores)
    ]
    res = run_bass_kernel_spmd(nc, in_maps, core_ids=list(range(n_cores)))
    return np.concatenate([res.results[c]["out"] for c in range(n_cores)], axis=0)


# revision 13
# speedup vs baseline: 1.0252x; 1.0252x over previous
"""MoE BasicBlock kernel v3 for TRN2: 1D Winograd F(2,3) along x.

W = sum_e alpha_e * w_e (21 experts), conv3x3 -> BN -> relu -> conv3x3 -> BN
-> +x -> relu on x [N,256,56,56] f32. Data-parallel across 8 cores (4
images/core); the alpha-combine is oc-sharded and shared via one fused bf16
AllGather per rep (as v2).

v3 replaces direct conv with 1D Winograd F(2,3) along the x axis, cutting PE
multiplies 1.5x (18 -> 12 effective K=128 matmul-equivalents per output):
  per output row-pair (x = 2t, 2t+1) and ky tap:
    d = [xp[2t], xp[2t+1], xp[2t+2], xp[2t+3]]  (padded cols)
    U0 = d0-d2, U1 = d1+d2, U2 = d1-d2, U3 = d1-d3         (moving, bf16)
    G0 = w0, G1 = (w0+w1+w2)/2, G2 = (w1-w0-w2)/2, G3 = w2 (stationary; G2
         carries the folded sign so psum slab nu=2 equals Lavin's m3)
    y0 = m0+m1+m2 ; y1 = m1-m2-m3                          (DVE, f32 psum)
Layout tricks:
  - activations live as even/odd padded-column planes ([128,58,29] bf16) so
    every U combine is a packed stride-1 read (DVE 2x eligible) and the
    conv2 residual add reads packed slices;
  - psum tiles are [128,2,512] f32 (two banks; each nu slab bank-aligned so
    interleaved accumulation groups never share a zero-region);
  - the expert combine is emitted as a generator pumped between row-tile
    blocks so no long DVE block starves psum drains;
  - U builds run on GPSIMD for conv1 and DVE for conv2; drains on DVE;
    epilogues on ACT.
"""

import numpy as np

import concourse.bass as bass
import concourse.mybir as mybir
import concourse.tile as tile
from concourse.masks import make_identity

FP32 = mybir.dt.float32
BF16 = mybir.dt.bfloat16

C = 256  # channels
CCH = 2  # channel chunks of 128
H = W = 56
HP = 58  # padded rows
PC = 29  # cols per even/odd plane
TW = 28  # winograd output col-pairs
E = 21  # experts
KHW = 9
IC9 = C * KHW  # 2304
RT = (14, 14, 14, 14)  # output rows per tile (56 = 4*14)
R0 = (0, 14, 28, 42)
NRT = 4
EPS = 1e-5
COPY = mybir.ActivationFunctionType.Copy
RELU = mybir.ActivationFunctionType.Relu
MUL = mybir.AluOpType.mult
ADD = mybir.AluOpType.add


def split_multi_waits(nc):
    """The installed walrus accepts at most one sync-wait per instruction
    (two for EventSemaphore). Tile's sem assignment can emit more; split the
    extras onto injected same-engine nops placed immediately before the
    offending instruction (equivalent semantics for in-order engine streams).
    """
    n_split = 0
    n_dma_split = 0
    for bb in nc.main_func.blocks:
        new_list = []
        for inst in list(bb.instructions):
            si = inst.sync_info
            waits = list(si.on_wait) if si is not None and si.on_wait else []
            cap = 2 if isinstance(inst, mybir.InstEventSemaphore) else 1
            if len(waits) > cap:
                if getattr(inst, "queue", None) is not None:
                    n_dma_split += 1
                extra, keep = waits[:-cap], waits[-cap:]
                for w in extra:
                    nop = nc.engines[inst.engine].nop(hint="waitsplit", nofuse=True)
                    host_bb = nc.cur_bb
                    assert host_bb.bb.instructions[-1] is nop.ins
                    host_bb.bb.instructions.pop()
                    nop.ins.sync_info = mybir.SyncInfo(on_update=[], on_wait=[w])
                    new_list.append(nop.ins)
                    n_split += 1
                inst.sync_info = mybir.SyncInfo(
                    on_update=list(si.on_update) if si.on_update else [], on_wait=keep
                )
            new_list.append(inst)
        bb.instructions[:] = new_list
    return n_split, n_dma_split


def build_nc(npc=4, repeat=1, n_cores=8):
    nc = bass.Bass(
        "TRN2", target_bir_lowering=False, debug=False, num_devices=n_cores
    )

    OSH = C // n_cores  # oc rows combined per core (32)
    SFREE = OSH * IC9 // 128  # 576

    x = nc.dram_tensor("x", [npc, C, H, W], FP32, kind="ExternalInput")
    alpha = nc.dram_tensor("alpha", [E], FP32, kind="ExternalInput")
    w1 = nc.dram_tensor("w1s", [E, OSH, C, 3, 3], FP32, kind="ExternalInput")
    w2 = nc.dram_tensor("w2s", [E, OSH, C, 3, 3], FP32, kind="ExternalInput")
    bn = {}
    for nm in ("g1", "b1", "m1", "v1", "g2", "b2", "m2", "v2"):
        bn[nm] = nc.dram_tensor(nm, [C], FP32, kind="ExternalInput")
    out = nc.dram_tensor("out", [npc, C, H, W], FP32, kind="ExternalOutput")

    xap = x.ap().rearrange("n c h w -> n c (h w)")
    oap = out.ap().rearrange("n c h w -> n c (h w)")
    w1ap = w1.ap().rearrange("e o i h w -> e (o i h w)")
    w2ap = w2.ap().rearrange("e o i h w -> e (o i h w)")
    wparts = [nc.dram_tensor(f"wpart{i}", [2 * OSH * IC9], BF16) for i in range(2)]
    wgaths = [
        nc.dram_tensor(f"wgath{i}", [n_cores, 2, OSH, IC9], BF16, addr_space="Shared")
        for i in range(2)
    ]

    with tile.TileContext(nc) as tc:
        import contextlib

        with contextlib.ExitStack() as ctx:
            singles = ctx.enter_context(tc.tile_pool(name="singles", bufs=1))
            epool = ctx.enter_context(tc.tile_pool(name="epool", bufs=3))
            wfpool = ctx.enter_context(tc.tile_pool(name="wfpool", bufs=2))
            accpool = ctx.enter_context(tc.tile_pool(name="accpool", bufs=2))
            xspool = ctx.enter_context(tc.tile_pool(name="xspool", bufs=2))
            xplanes = ctx.enter_context(tc.tile_pool(name="xplanes", bufs=4 * npc))
            hplanes = ctx.enter_context(tc.tile_pool(name="hplanes", bufs=8))
            upool = ctx.enter_context(tc.tile_pool(name="upool", bufs=6))
            ypool = ctx.enter_context(tc.tile_pool(name="ypool", bufs=4))
            ytpool = ctx.enter_context(tc.tile_pool(name="ytpool", bufs=8))
            rrpool = ctx.enter_context(tc.tile_pool(name="rrpool", bufs=4))
            wscr = ctx.enter_context(tc.tile_pool(name="wscr", bufs=4))
            obpool = ctx.enter_context(tc.tile_pool(name="obpool", bufs=2))
            cpsum = ctx.enter_context(tc.tile_pool(name="cpsum", bufs=3, space="PSUM"))
            tpsum = ctx.enter_context(tc.tile_pool(name="tpsum", bufs=2, space="PSUM"))

            # ---- stage 0: BN params, alpha, identity ----
            ident = singles.tile([128, 128], BF16, tag="ident")
            make_identity(nc, ident[:])

            zero_c = singles.tile([128, 1], FP32, tag="zero_c")
            nc.vector.memset(zero_c[:], 0.0)
            nc.const_aps.aps[(FP32, 0.0)] = zero_c[:]
            eps_c = singles.tile([128, 1], FP32, tag="eps_c")
            nc.vector.memset(eps_c[:], EPS)

            alpha_sb = singles.tile([128, E], FP32, tag="alpha")
            nc.sync.dma_start(
                out=alpha_sb[:],
                in_=bass.AP(tensor=alpha.ap().tensor, offset=0, ap=[[0, 128], [1, E]]),
            )

            bns = {}
            for nm in ("g1", "b1", "m1", "v1", "g2", "b2", "m2", "v2"):
                t = singles.tile([128, CCH], FP32, name=f"bn_{nm}", tag=f"bn_{nm}")
                nc.sync.dma_start(
                    out=t[:],
                    in_=bass.AP(
                        tensor=bn[nm].ap().tensor, offset=0, ap=[[1, 128], [128, CCH]]
                    ),
                )
                bns[nm] = t

            def bn_fold(g, b, m, v, idx):
                sq = singles.tile([128, CCH], FP32, name=f"bn_sq{idx}", tag=f"bn_sq{idx}")
                nc.scalar.activation(
                    sq[:], v[:], mybir.ActivationFunctionType.Sqrt, bias=eps_c[:]
                )
                r = singles.tile([128, CCH], FP32, name=f"bn_r{idx}", tag=f"bn_r{idx}")
                nc.vector.reciprocal(r[:], sq[:])
                ve = singles.tile([128, CCH], FP32, name=f"bn_ve{idx}", tag=f"bn_ve{idx}")
                nc.vector.tensor_scalar_add(ve[:], v[:], EPS)
                t1 = singles.tile([128, CCH], FP32, name=f"bn_t1{idx}", tag=f"bn_t1{idx}")
                nc.vector.tensor_mul(t1[:], ve[:], r[:])
                nc.vector.tensor_add(t1[:], t1[:], sq[:])
                nc.vector.tensor_scalar_mul(t1[:], t1[:], 0.5)
                nc.vector.reciprocal(r[:], t1[:])
                s = singles.tile([128, CCH], FP32, name=f"bn_s{idx}", tag=f"bn_s{idx}")
                nc.vector.tensor_mul(s[:], g[:], r[:])
                bp = singles.tile([128, CCH], FP32, name=f"bn_bp{idx}", tag=f"bn_bp{idx}")
                nc.vector.tensor_mul(bp[:], m[:], s[:])
                nc.vector.tensor_sub(bp[:], b[:], bp[:])
                return s, bp

            s1, b1p = bn_fold(bns["g1"], bns["b1"], bns["m1"], bns["v1"], 1)
            s2, b2p = bn_fold(bns["g2"], bns["b2"], bns["m2"], bns["v2"], 2)

            # G-transformed stationary: [128ic, 3ky, 4nu, 128oc] bf16
            lhsT = [
                [
                    [
                        [
                            singles.tile(
                                [128, 3, 4, 128], BF16,
                                name=f"lhsT_{par}_{wi}_{ic}_{oc}",
                                tag=f"lhsT_{par}_{wi}_{ic}_{oc}",
                            )
                            for oc in range(CCH)
                        ]
                        for ic in range(CCH)
                    ]
                    for wi in range(2)
                ]
                for par in range(2)
            ]

            # ---- pumped weight combine + one fused bf16 AllGather ----
            gens = []

            def pump():
                while gens:
                    try:
                        next(gens[0])
                        return
                    except StopIteration:
                        gens.pop(0)

            def drain():
                while gens:
                    try:
                        next(gens[0])
                    except StopIteration:
                        gens.pop(0)

            def combine_gather_gen(par):
                accb = accpool.tile([128, 2 * SFREE], BF16, name="accb", tag="accb")
                for wi, wap in ((0, w1ap), (1, w2ap)):
                    acc = wfpool.tile([128, SFREE], FP32, name="sacc", tag="sacc")
                    for e0 in range(0, E, 3):
                        for e in range(e0, min(e0 + 3, E)):
                            est = epool.tile([128, SFREE], FP32, name="sest", tag="sest")
                            nc.sync.dma_start(
                                out=est[:],
                                in_=wap[e].rearrange("(p f) -> p f", p=128),
                            )
                            if e == 0:
                                nc.vector.tensor_scalar_mul(
                                    acc[:], est[:], alpha_sb[:, 0:1]
                                )
                            else:
                                nc.vector.scalar_tensor_tensor(
                                    acc[:], est[:], alpha_sb[:, e : e + 1], acc[:],
                                    op0=MUL, op1=ADD,
                                )
                        yield
                    nc.scalar.copy(
                        out=accb[:, wi * SFREE : (wi + 1) * SFREE], in_=acc[:]
                    )
                    yield
                nc.sync.dma_start(
                    out=wparts[par].ap().rearrange("(w p f) -> p w f", w=2, p=128),
                    in_=accb[:].rearrange("p (w f) -> p w f", w=2),
                )
                nc.gpsimd.collective_compute(
                    "AllGather",
                    mybir.AluOpType.bypass,
                    replica_groups=[list(range(n_cores))],
                    ins=[wparts[par].ap().opt()],
                    outs=[wgaths[par].ap().rearrange("c w o f -> (c w o f)").opt()],
                )

            def load_transpose(par, wi):
                # gathered rows of weight wi live at wgath[4*oc+k, wi, :, :]
                for oc in range(CCH):
                    wf = wfpool.tile([128, IC9], BF16, name="wfull", tag="wfull")
                    for k in range(4):
                        nc.sync.dma_start(
                            out=wf[32 * k : 32 * (k + 1), :],
                            in_=wgaths[par].ap()[4 * oc + k, wi],
                        )
                    wfr = wf[:].rearrange("p (c i r) -> p c i r", c=CCH, r=KHW)
                    for ic in range(CCH):
                        dst = lhsT[par][wi][ic][oc]
                        for ky in range(3):
                            # transpose the three kx taps, then G-combine:
                            # nu0 = w0, nu3 = w2, s1h = w1/2,
                            # nu1 = (w0+w2)/2 + s1h, nu2 = -(w0+w2)/2 + s1h
                            pt = tpsum.tile([128, 128], BF16, name="tp", tag="tp")
                            nc.tensor.transpose(
                                pt[:], wfr[:, ic, :, 3 * ky + 0], ident[:]
                            )
                            nc.scalar.copy(out=dst[:, ky, 0, :], in_=pt[:])
                            pt = tpsum.tile([128, 128], BF16, name="tp", tag="tp")
                            nc.tensor.transpose(
                                pt[:], wfr[:, ic, :, 3 * ky + 1], ident[:]
                            )
                            s1h = wscr.tile([128, 128], BF16, name="ws", tag="ws")
                            nc.scalar.activation(s1h[:], pt[:], COPY, scale=0.5)
                            pt = tpsum.tile([128, 128], BF16, name="tp", tag="tp")
                            nc.tensor.transpose(
                                pt[:], wfr[:, ic, :, 3 * ky + 2], ident[:]
                            )
                            nc.scalar.copy(out=dst[:, ky, 3, :], in_=pt[:])
                            qh = wscr.tile([128, 128], BF16, name="ws", tag="ws")
                            nc.vector.tensor_add(
                                qh[:], dst[:, ky, 0, :], dst[:, ky, 3, :]
                            )
                            nc.vector.scalar_tensor_tensor(
                                dst[:, ky, 1, :], qh[:], 0.5, s1h[:],
                                op0=MUL, op1=ADD,
                            )
                            nc.vector.scalar_tensor_tensor(
                                dst[:, ky, 2, :], qh[:], -0.5, s1h[:],
                                op0=MUL, op1=ADD,
                            )

            # ---- even/odd padded-column planes ----
            def fill_planes(n):
                planes = []
                for c in range(CCH):
                    xe = xplanes.tile([128, HP, PC], BF16, name="xpl", tag="xpl")
                    xo = xplanes.tile([128, HP, PC], BF16, name="xpl", tag="xpl")
                    nc.gpsimd.memset(xe[:, 0, :], 0.0)
                    nc.gpsimd.memset(xe[:, HP - 1, :], 0.0)
                    nc.gpsimd.memset(xe[:, 1 : HP - 1, 0:1], 0.0)
                    nc.gpsimd.memset(xo[:, 0, :], 0.0)
                    nc.gpsimd.memset(xo[:, HP - 1, :], 0.0)
                    nc.gpsimd.memset(xo[:, 1 : HP - 1, PC - 1 : PC], 0.0)
                    for half in range(2):
                        xst = xspool.tile([128, 28 * W], FP32, name="xst", tag="xst")
                        nc.sync.dma_start(
                            out=xst[:],
                            in_=xap[
                                n, c * 128 : (c + 1) * 128,
                                half * 28 * W : (half + 1) * 28 * W,
                            ],
                        )
                        xsr = xst[:].rearrange("p (r t two) -> p r t two", r=28, two=2)
                        r0 = 1 + 28 * half
                        nc.scalar.copy(
                            out=xe[:, r0 : r0 + 28, 1:PC], in_=xsr[:, :, :, 1]
                        )
                        nc.scalar.copy(
                            out=xo[:, r0 : r0 + 28, 0 : PC - 1], in_=xsr[:, :, :, 0]
                        )
                    planes.append((xe, xo))
                return planes

            def alloc_hplanes():
                sets = []
                for c in range(CCH):
                    he = hplanes.tile([128, HP, PC], BF16, name="hpl", tag="hpl")
                    ho = hplanes.tile([128, HP, PC], BF16, name="hpl", tag="hpl")
                    nc.gpsimd.memset(he[:, 0, :], 0.0)
                    nc.gpsimd.memset(he[:, HP - 1, :], 0.0)
                    nc.gpsimd.memset(he[:, 1 : HP - 1, 0:1], 0.0)
                    nc.gpsimd.memset(ho[:, 0, :], 0.0)
                    nc.gpsimd.memset(ho[:, HP - 1, :], 0.0)
                    nc.gpsimd.memset(ho[:, 1 : HP - 1, PC - 1 : PC], 0.0)
                    sets.append((he, ho))
                return sets

            def build_u(src, ic, rt, eng):
                r0 = R0[rt]
                rows = RT[rt] + 2
                pe_, po = src[ic]
                u = upool.tile([128, 4, 16, 28], BF16, name="u", tag="u")
                a = pe_[:, r0 : r0 + rows, 0:28]
                b = pe_[:, r0 : r0 + rows, 1:29]
                cc = po[:, r0 : r0 + rows, 0:28]
                dd = po[:, r0 : r0 + rows, 1:29]
                eng.tensor_sub(u[:, 0, :rows, :], a, b)
                eng.tensor_add(u[:, 1, :rows, :], cc, b)
                eng.tensor_sub(u[:, 2, :rows, :], cc, b)
                eng.tensor_sub(u[:, 3, :rows, :], cc, dd)
                return u

            def conv(n, src, wi, dst, par):
                """One Winograd conv over image n. src: per-chunk (even, odd)
                planes. dst: wi=0 -> hplane sets; wi=1 -> x planes (residual),
                output DMA'd."""
                eng_u = nc.gpsimd if wi == 0 else nc.vector
                us = [build_u(src, ic, 0, eng_u) for ic in range(CCH)]
                for rt in range(NRT):
                    rows = RT[rt]
                    r0 = R0[rt]
                    fcols = rows * TW
                    cur = us
                    for oc in range(CCH):
                        ps01 = cpsum.tile([128, 2, 512], FP32, name="cp", tag="cp")
                        ps23 = cpsum.tile([128, 2, 512], FP32, name="cp", tag="cp")
                        for ic in range(CCH):
                            for ky in range(3):
                                for nu in range(4):
                                    pst = ps01 if nu < 2 else ps23
                                    nc.tensor.matmul(
                                        pst[:, nu % 2, 0:fcols],
                                        lhsT[par][wi][ic][oc][:, ky, nu, :],
                                        cur[ic][:, nu, ky : ky + rows, :],
                                        start=(ic == 0 and ky == 0),
                                        stop=(ic == 1 and ky == 2),
                                    )
                        if oc == 0 and rt + 1 < NRT:
                            # next row-tile's U ahead of the drains in the
                            # u-engine stream
                            us = [
                                build_u(src, ic, rt + 1, eng_u) for ic in range(CCH)
                            ]
                        m0 = ps01[:, 0, 0:fcols]
                        m1 = ps01[:, 1, 0:fcols]
                        m2 = ps23[:, 0, 0:fcols]
                        m3 = ps23[:, 1, 0:fcols]
                        y0 = ypool.tile([128, 14, 28], BF16, name="y", tag="y")
                        y1 = ypool.tile([128, 14, 28], BF16, name="y", tag="y")
                        c0 = ytpool.tile([128, 14, 28], BF16, name="yt", tag="yt")
                        c2 = ytpool.tile([128, 14, 28], BF16, name="yt", tag="yt")
                        t0 = ytpool.tile([128, 14, 28], BF16, name="yt", tag="yt")
                        t1 = ytpool.tile([128, 14, 28], BF16, name="yt", tag="yt")
                        y0f = y0[:].rearrange("p r c -> p (r c)")[:, 0:fcols]
                        y1f = y1[:].rearrange("p r c -> p (r c)")[:, 0:fcols]
                        c0f = c0[:].rearrange("p r c -> p (r c)")[:, 0:fcols]
                        c2f = c2[:].rearrange("p r c -> p (r c)")[:, 0:fcols]
                        t0f = t0[:].rearrange("p r c -> p (r c)")[:, 0:fcols]
                        t1f = t1[:].rearrange("p r c -> p (r c)")[:, 0:fcols]
                        # TensorTensor may read only one PSUM operand: stage
                        # m1/m2 to SBUF on ACT so two of the four DVE combines
                        # run on packed bf16 SBUF operands (2x eligible).
                        nc.scalar.copy(out=c0f, in_=m1)
                        nc.scalar.copy(out=c2f, in_=m2)
                        nc.vector.tensor_add(t0f, m0, c0f)
                        nc.vector.tensor_add(y0f, t0f, c2f)
                        nc.vector.tensor_sub(t1f, c0f, c2f)
                        nc.vector.tensor_sub(y1f, t1f, m3)
                        if wi == 0:
                            he, ho = dst[oc]
                            nc.scalar.activation(
                                ho[:, r0 + 1 : r0 + 1 + rows, 0:28],
                                y0[:, :rows, :], RELU,
                                bias=b1p[:, oc : oc + 1], scale=s1[:, oc : oc + 1],
                            )
                            nc.scalar.activation(
                                he[:, r0 + 1 : r0 + 1 + rows, 1:29],
                                y1[:, :rows, :], RELU,
                                bias=b1p[:, oc : oc + 1], scale=s1[:, oc : oc + 1],
                            )
                        else:
                            xe, xo = dst[oc]
                            rr0 = rrpool.tile([128, 14, 28], BF16, name="rr", tag="rr")
                            rr1 = rrpool.tile([128, 14, 28], BF16, name="rr", tag="rr")
                            nc.vector.scalar_tensor_tensor(
                                rr0[:, :rows, :], y0[:, :rows, :],
                                s2[:, oc : oc + 1],
                                xo[:, r0 + 1 : r0 + 1 + rows, 0:28],
                                op0=MUL, op1=ADD,
                            )
                            nc.vector.scalar_tensor_tensor(
                                rr1[:, :rows, :], y1[:, :rows, :],
                                s2[:, oc : oc + 1],
                                xe[:, r0 + 1 : r0 + 1 + rows, 1:29],
                                op0=MUL, op1=ADD,
                            )
                            ob = obpool.tile([128, 14, 56], FP32, name="ob", tag="ob")
                            obv = ob[:].rearrange("p r (t two) -> p r t two", two=2)
                            nc.scalar.activation(
                                obv[:, :rows, :, 0], rr0[:, :rows, :], RELU,
                                bias=b2p[:, oc : oc + 1],
                            )
                            nc.scalar.activation(
                                obv[:, :rows, :, 1], rr1[:, :rows, :], RELU,
                                bias=b2p[:, oc : oc + 1],
                            )
                            nc.sync.dma_start(
                                out=oap[
                                    n, oc * 128 : (oc + 1) * 128,
                                    r0 * W : (r0 + rows) * W,
                                ],
                                in_=ob[:].rearrange("p r c -> p (r c)")[
                                    :, 0 : rows * W
                                ],
                            )
                    pump()

            # ---- emission schedule ----
            assert npc == 4
            gens.append(combine_gather_gen(0))
            drain()
            load_transpose(0, 0)
            load_transpose(0, 1)
            for rep in range(repeat):
                par = rep % 2
                if rep + 1 < repeat:
                    gens.append(combine_gather_gen(1 - par))
                xps = {0: fill_planes(0), 1: fill_planes(1)}
                hps = {}
                hps[0] = alloc_hplanes()
                conv(0, xps[0], 0, hps[0], par)
                xps[2] = fill_planes(2)
                hps[1] = alloc_hplanes()
                conv(1, xps[1], 0, hps[1], par)
                conv(0, hps[0], 1, xps[0], par)
                xps[3] = fill_planes(3)
                hps[2] = alloc_hplanes()
                conv(2, xps[2], 0, hps[2], par)
                drain()
                if rep + 1 < repeat:
                    load_transpose(1 - par, 0)
                conv(1, hps[1], 1, xps[1], par)
                hps[3] = alloc_hplanes()
                conv(3, xps[3], 0, hps[3], par)
                if rep + 1 < repeat:
                    load_transpose(1 - par, 1)
                conv(2, hps[2], 1, xps[2], par)
                conv(3, hps[3], 1, xps[3], par)

    n_split, n_dma_split = split_multi_waits(nc)
    return nc, (n_split, n_dma_split)


# ---------------------------------------------------------------------------
# Host-side entry point: takes FULL inputs, shards batch across 8 cores.
# ---------------------------------------------------------------------------
_NC_CACHE = {}


def kernel(**inputs):
    from concourse.bass_utils import run_bass_kernel_spmd

    x = np.ascontiguousarray(np.asarray(inputs["x"], dtype=np.float32))
    n_total = x.shape[0]
    n_cores = 8
    npc = n_total // n_cores
    assert npc * n_cores == n_total

    key = npc
    if key not in _NC_CACHE:
        _NC_CACHE[key] = build_nc(npc=npc)[0]
    nc = _NC_CACHE[key]

    w1 = np.asarray(inputs["w1"], dtype=np.float32)
    w2 = np.asarray(inputs["w2"], dtype=np.float32)
    osh = w1.shape[1] // n_cores
    shared = {
        k: np.ascontiguousarray(np.asarray(v, dtype=np.float32))
        for k, v in inputs.items()
        if k not in ("x", "w1", "w2")
    }
    in_maps = [
        {
            "x": x[c * npc : (c + 1) * npc],
            "w1s": np.ascontiguousarray(w1[:, c * osh : (c + 1) * osh]),
            "w2s": np.ascontiguousarray(w2[:, c * osh : (c + 1) * osh]),
            **shared,
        }
        for c in range(n_cores)
    ]
    res = run_bass_kernel_spmd(nc, in_maps, core_ids=list(range(n_cores)))
    return np.concatenate([res.results[c]["out"] for c in range(n_cores)], axis=0)
